# revision 32
# baseline (speedup 1.0000x reference)
"""Dark-Channel-Prior dehazing (DCPGenerator) Trainium2 Bass kernel.

Contract: kernel(x: [16,3,512,512] f32) -> [16,3,512,512] f32.
Data-parallel over 8 NeuronCores: 2 samples per core. Each core runs the
full per-sample pipeline on-device:
  guidance/img prep -> dark channel (15x15 min-pool, bf16) -> atmospheric
  light (top-1% selection via secant-estimated threshold + band-corrected
  mean, bf16 counting) -> second dark channel on img/A (bf16) -> guided
  filter (r=40 box sums via free-dim scans + fp32r banded-matmul partition
  sums) -> output.
"""
import numpy as np
from contextlib import ExitStack

H = 512
W = 512
NCHUNK = 4          # 4 row-chunks of 128 partitions
CW = 512            # chunk free width
PADW = 526          # padded chunk width for the 15-wide min pool (7+512+7)
CUMW = 593          # hbox cum chunk: 41 zeros | 512 cumsum | 40 x cum[511]
WIN_PAD = 7
RADIUS = 40
EPS = 1e-3
OMEGA = 0.95
TOPN = int(0.01 * H * W)          # 2621
T0 = 0.0055                       # secant bracket on raw-x dark scale
T1 = 0.0085
BAND = 2e-4                       # band width for tie-region correction
SECANT_ROUNDS = 6

_CACHE = {}


# ---------------------------------------------------------------- host consts
def _host_consts():
    n1 = np.minimum(np.arange(H) + RADIUS, H - 1) - np.maximum(np.arange(H) - RADIUS, 0) + 1
    inv_nh = (1.0 / n1).astype(np.float32)          # [512]
    inv_nw = inv_nh.copy()                          # same for W=512
    invnh = np.zeros((128, NCHUNK), np.float32)
    for c in range(NCHUNK):
        invnh[:, c] = inv_nh[c * 128:(c + 1) * 128]
    invnw_rep = np.broadcast_to(inv_nw[None, :], (128, W)).copy()
    k = np.arange(128)[:, None]
    p = np.arange(128)[None, :]
    band = (np.abs(k - p) <= RADIUS).astype(np.float32)
    bu = (k >= p + 128 - RADIUS).astype(np.float32) / 81.0
    bd = (k <= p - (128 - RADIUS)).astype(np.float32) / 81.0
    bms = []
    for c in range(NCHUNK):
        bms.append(band * inv_nh[c * 128:(c + 1) * 128][None, :] / 81.0)
    bu = bu / 81.0
    bd = bd / 81.0
    # 81*invnw fix factors for the 40 edge columns each side (1.0 interior)
    fixl = np.broadcast_to((81.0 * inv_nw[0:RADIUS])[None, :], (128, RADIUS)).copy()
    fixr = np.broadcast_to((81.0 * inv_nw[W - RADIUS:])[None, :], (128, RADIUS)).copy()
    return {"invnh": invnh, "invnw": invnw_rep, "fixl": fixl, "fixr": fixr,
            "bm0": bms[0], "bm1": bms[1], "bm3": bms[3], "bu": bu, "bd": bd}


# ------------------------------------------------------------------ program
def _build():
    import concourse.bacc as bacc
    import concourse.tile as tile
    import concourse.bass as bass
    from concourse import mybir

    f32 = mybir.dt.float32
    f32r = mybir.dt.float32r
    bf16 = mybir.dt.bfloat16
    Alu = mybir.AluOpType
    Act = mybir.ActivationFunctionType

    nc = bacc.Bacc("TRN2", target_bir_lowering=False, debug=False, num_devices=8)

    x_ext = nc.dram_tensor("x", [2, 3, H, W], f32, kind="ExternalInput").ap()
    band_exts = {nm: nc.dram_tensor(nm, [128, 128], f32, kind="ExternalInput").ap()
                 for nm in ("bm0", "bm1", "bm3", "bu", "bd")}
    invnh_ext = nc.dram_tensor("invnh", [128, NCHUNK], f32, kind="ExternalInput").ap()
    invnw_ext = nc.dram_tensor("invnw", [128, W], f32, kind="ExternalInput").ap()
    fixl_ext = nc.dram_tensor("fixl", [128, RADIUS], f32, kind="ExternalInput").ap()
    fixr_ext = nc.dram_tensor("fixr", [128, RADIUS], f32, kind="ExternalInput").ap()
    y_ext = nc.dram_tensor("y", [2, 3, H, W], f32, kind="ExternalOutput").ap()

    def cview(t, width=CW):
        """[128, NCHUNK*width] tile -> [128, NCHUNK, width] view."""
        return t.rearrange("p (c w) -> p c w", w=width)

    def fbcast(ap_col, n):
        """free-dim step-0 broadcast of a [...,1] AP to [...,n]."""
        return bass.AP(tensor=ap_col.tensor, offset=ap_col.offset,
                       ap=[list(p) for p in ap_col.ap[:-1]] + [[0, n]])

    with ExitStack() as ctx:
        tc = ctx.enter_context(tile.TileContext(nc))

        cpool = ctx.enter_context(tc.tile_pool(name="cpool", bufs=1))
        big = ctx.enter_context(tc.tile_pool(name="big", bufs=1))
        pp = ctx.enter_context(tc.tile_pool(name="pp", bufs=1))       # minpool / box scratch
        boxes = ctx.enter_context(tc.tile_pool(name="boxes", bufs=5))
        srcp = ctx.enter_context(tc.tile_pool(name="srcp", bufs=3))
        abt = ctx.enter_context(tc.tile_pool(name="abt", bufs=3))
        tiny = ctx.enter_context(tc.tile_pool(name="tiny", bufs=2))
        pbig = ctx.enter_context(tc.tile_pool(name="pbig", bufs=2, space="PSUM"))
        psml = ctx.enter_context(tc.tile_pool(name="psml", bufs=2, space="PSUM"))

        # ---- constants ----
        c_band = {}
        stage = cpool.tile([128, 128], f32, name="s_band")
        for nm in ("bm0", "bm1", "bm3", "bu", "bd"):
            nc.sync.dma_start(out=stage[:], in_=band_exts[nm][:])
            c_band[nm] = cpool.tile([128, 128], f32r, name=f"c_{nm}")
            nc.scalar.copy(c_band[nm][:], stage[:])
        c_bm = [c_band["bm0"], c_band["bm1"], c_band["bm1"], c_band["bm3"]]
        c_invnh = cpool.tile([128, NCHUNK], f32, name="c_invnh")
        nc.sync.dma_start(out=c_invnh[:], in_=invnh_ext[:])
        c_invnw = cpool.tile([128, W], f32, name="c_invnw")
        nc.sync.dma_start(out=c_invnw[:], in_=invnw_ext[:])
        c_fixl = cpool.tile([128, RADIUS], f32, name="c_fixl")
        nc.sync.dma_start(out=c_fixl[:], in_=fixl_ext[:])
        c_fixr = cpool.tile([128, RADIUS], f32, name="c_fixr")
        nc.sync.dma_start(out=c_fixr[:], in_=fixr_ext[:])
        c_ones128 = cpool.tile([128, 1], f32, name="c_ones128")
        nc.vector.memset(c_ones128[:], 1.0)
        c_ones1x = cpool.tile([1, 128], f32, name="c_ones1x")
        nc.vector.memset(c_ones1x[:], 1.0)
        c_zeros = cpool.tile([128, CW], f32, name="c_zeros")
        nc.vector.memset(c_zeros[:], 0.0)
        c_ones16 = cpool.tile([128, CW], bf16, name="c_ones16")
        nc.vector.memset(c_ones16[:], 1.0)
        c_e07 = cpool.tile([128, 7], bf16, name="c_e07")
        nc.vector.memset(c_e07[:], 0.0)
        nc.vector.memset(c_e07[0:1, :], 1.0)

        # ------------------------------------------------ helpers (emit ops)
        def interior(t):
            """padded tile -> [128, NCHUNK, CW] strided view of the interiors."""
            return cview(t, PADW)[:, :, WIN_PAD:WIN_PAD + CW]

        def memset_pads(t, eng):
            v = cview(t, PADW)
            for c in range(NCHUNK):
                eng.memset(v[:, c, 0:WIN_PAD], 1.0)
                eng.memset(v[:, c, PADW - WIN_PAD:PADW], 1.0)

        def hpool(dst, padded, w1, eng):
            """15-wide sliding min along free dim; padded [128,4*526] -> dst [128,4*512].
            Single multi-dim-AP instructions across all 4 chunks."""
            a = cview(padded, PADW)
            b = cview(w1, PADW)
            d = cview(dst)
            eng.tensor_tensor(b[:, :, 0:525], a[:, :, 0:525], a[:, :, 1:526], Alu.min)
            eng.tensor_tensor(a[:, :, 0:523], b[:, :, 0:523], b[:, :, 2:525], Alu.min)
            eng.tensor_tensor(b[:, :, 0:519], a[:, :, 0:519], a[:, :, 4:523], Alu.min)
            eng.tensor_tensor(d[:, 0:NCHUNK, :], b[:, :, 0:512], b[:, :, 7:519],
                              Alu.min)

        def vshift_dma(dst, src, s, ring):
            """dst[row r] = src[row r+s] (global 512-row space). src has a
            5th all-ones chunk so the bottom-pad rows ride the wrap DMA."""
            dv, sv = cview(dst), cview(src)
            ring.dma_start(out=dv[0:128 - s, :, :], in_=sv[s:128, 0:NCHUNK, :])
            ring.dma_start(out=dv[128 - s:128, :, :], in_=sv[0:s, 1:NCHUNK + 1, :])

        def vshift_dma_down(dst, src, s, ring):
            """dst[row r] = src[max(r-s, 0)]; the top-clamp rows of chunk 0 are
            NOT written here — the caller patches them via a PE broadcast."""
            dv, sv = cview(dst), cview(src)
            ring.dma_start(out=dv[s:128, :, :], in_=sv[0:128 - s, 0:NCHUNK, :])
            ring.dma_start(out=dv[0:s, 1:NCHUNK, :],
                           in_=sv[128 - s:128, 0:NCHUNK - 1, :])

        def hbox(dst, src, cum, eng_scan, eng_elem):
            """zero-padded 81-wide box sum along free dim. src [128,2048];
            dst [128,2048] (f32r); cum [128, 4*593] with per-chunk layout
            [41 zeros|512 cum|40 rep] (zeros pre-set once per sample)."""
            sv, dv, cv = cview(src), cview(dst), cview(cum, CUMW)
            for c in range(NCHUNK):
                eng_scan.tensor_tensor_scan(cv[:, c, 41:553], sv[:, c, :], c_zeros[:],
                                            0.0, Alu.add, Alu.add)
            for c in range(NCHUNK):
                eng_elem.tensor_copy(cv[:, c, 553:593], fbcast(cv[:, c, 552:553], 40))
            eng_elem.tensor_tensor(dv[:, :, :], cv[:, :, 81:593], cv[:, :, 0:512],
                                   Alu.subtract)

        def vbox(dst, src, eng=None):
            """mean over the 81-tall zero-padded vertical box. Band matrices
            carry invNh/81; PSUM evacuated by the Scalar engine (plain copy),
            then V fixes the 40 edge columns each side with 81*invNw."""
            sv, dv = cview(src), cview(dst)
            for c in range(NCHUNK):
                ops = []
                if c > 0:
                    ops.append((c_band["bu"], c - 1))
                ops.append((c_bm[c], c))
                if c < NCHUNK - 1:
                    ops.append((c_band["bd"], c + 1))
                ps = pbig.tile([128, CW], f32, name="vps", tag="vps")
                for i, (mat, sc_) in enumerate(ops):
                    nc.tensor.matmul(ps[:], mat[:], sv[:, sc_, :],
                                     start=(i == 0), stop=(i == len(ops) - 1))
                nc.scalar.copy(dv[:, c, :], ps[:])
            nc.vector.tensor_tensor(dv[:, :, 0:RADIUS], dv[:, :, 0:RADIUS],
                                    c_fixl[:].unsqueeze(1).broadcast_to(
                                        [128, NCHUNK, RADIUS]), Alu.mult)
            nc.vector.tensor_tensor(dv[:, :, CW - RADIUS:CW],
                                    dv[:, :, CW - RADIUS:CW],
                                    c_fixr[:].unsqueeze(1).broadcast_to(
                                        [128, NCHUNK, RADIUS]), Alu.mult)

        # ======================================================== per sample
        for s in range(2):
            V = nc.vector
            G = nc.vector  # gpsimd measured ~6x slower on full tiles; keep off path

            # ---- load (one DMA per channel via 3D AP) ----
            xch = []
            for chn in range(3):
                t = big.tile([128, NCHUNK * CW], f32, name=f"x{chn}", tag=f"x{chn}")
                nc.scalar.dma_start(out=cview(t)[:, :, :],
                                    in_=x_ext[s, chn].rearrange("(c p) w -> p c w",
                                                                p=128))
                xch.append(t)
            xr, xg, xb = xch

            # ---- bf16 channel copies (for dark1 + masked sums) ----
            x16 = []
            for chn, xt in enumerate(xch):
                t16 = srcp.tile([128, NCHUNK * CW], bf16, name=f"x16_{chn}",
                                tag="srcp")
                nc.scalar.activation(t16[:], xt[:], Act.Copy, bias=0.0, scale=1.0)
                x16.append(t16)
            xr16, xg16, xb16 = x16

            # ---- guidance I = ((.2989 xr + .587 xg + .114 xb) + 1)/2 (f32) ----
            Ia = pp.tile([128, NCHUNK * CW], bf16, name="Ia", tag="shv")
            Ib = pp.tile([128, NCHUNK * CW], bf16, name="Ib", tag="Ib")
            nc.scalar.activation(Ib[:], xr[:], Act.Copy, bias=0.5, scale=0.14945)
            V.scalar_tensor_tensor(Ia[:], xg16[:], 0.2935, Ib[:], Alu.mult, Alu.add)
            V.scalar_tensor_tensor(Ib[:], xb16[:], 0.057, Ia[:], Alu.mult, Alu.add)
            guid = Ib  # final guidance lives in Ib; Ia free for reuse

            # ---- dark1 = minpool15(min_c x) in bf16 ----
            mxp = pp.tile([128, NCHUNK * PADW], bf16, name="mxp", tag="mxp")
            w1 = pp.tile([128, NCHUNK * PADW], bf16, name="w1", tag="w1")
            memset_pads(mxp, V)
            V.tensor_tensor(interior(mxp), cview(xr16)[:, :, :],
                            cview(xg16)[:, :, :], Alu.min)
            V.tensor_tensor(interior(mxp), interior(mxp),
                            cview(xb16)[:, :, :], Alu.min)
            uh = pp.tile([128, (NCHUNK + 1) * CW], bf16, name="uh", tag="uh")
            V.memset(cview(uh)[:, NCHUNK, :], 1.0)
            hpool(uh, mxp, w1, V)
            sh = pp.tile([128, NCHUNK * CW], bf16, name="sh", tag="shv")
            u2 = pp.tile([128, (NCHUNK + 1) * CW], bf16, name="u2", tag="u2")
            V.memset(cview(u2)[:, NCHUNK, :], 1.0)
            NW = NCHUNK * CW

            def cmin(dst_t, a_t2, b_t2, eng=V):
                eng.tensor_tensor(dst_t[:, 0:NW], a_t2[:, 0:NW], b_t2[:, 0:NW],
                                  Alu.min)

            def clamp_fix(dst_t, src_t):
                """dst[0:7, chunk0] = min(src[0:7, chunk0], src[row0, chunk0])."""
                bc = pbig.tile([7, CW], f32, name="clamp_ps", tag="clamp")
                nc.tensor.matmul(bc[:], c_e07[:], cview(src_t)[:, 0, :],
                                 start=True, stop=True)
                V.tensor_tensor(cview(dst_t)[0:7, 0, :], cview(src_t)[0:7, 0, :],
                                bc[:], Alu.min)

            vshift_dma(sh, uh, 1, nc.sync)
            cmin(u2, uh, sh)
            vshift_dma(sh, u2, 2, nc.sync)
            cmin(uh, u2, sh)
            vshift_dma(sh, uh, 4, nc.sync)
            cmin(u2, uh, sh)
            vshift_dma_down(sh, u2, 7, nc.sync)
            u = uh
            cmin(u, u2, sh)
            clamp_fix(u, u2)

            # ---- atmospheric light (bf16 counting / masked sums) ----
            junk = pp.tile([128, NCHUNK * CW], bf16, name="junk", tag="mxp")
            acc8 = tiny.tile([128, 8], f32, name="acc8", tag="acc8")
            V.memset(acc8[:], 0.0)
            thr = tiny.tile([128, 1], f32, name="thr", tag="thr")
            scal = tiny.tile([1, 16], f32, name="scal", tag="scal")
            V.memset(scal[:], 0.0)
            # scal cols: 0 ta, 1 Ca, 2 tb, 3 Cb, 4..temp
            V.memset(scal[:, 0:1], T0)
            V.memset(scal[:, 2:3], T1)

            uv = cview(u)

            def count_into(col, sub=False):
                if sub:
                    # chunks {0,2}, stride-2 cols: 1/4 of the pixels
                    V.tensor_scalar(cview(junk)[:, 0:2, 0:256],
                                    uv[:, 0:NCHUNK:2, 0:CW:2], thr[:], 0.0,
                                    Alu.is_gt, Alu.add,
                                    accum_out=acc8[:, col:col + 1])
                else:
                    V.tensor_scalar(junk[:], u[:, 0:NW], thr[:], 0.0,
                                    Alu.is_gt, Alu.add,
                                    accum_out=acc8[:, col:col + 1])
                fps = psml.tile([1, 1], f32, name="fold_ps", tag="fold_ps")
                nc.tensor.matmul(fps[:], c_ones128[:], acc8[:, col:col + 1],
                                 start=True, stop=True)
                return fps

            def bcast_thr(src_col):
                bp = psml.tile([128, 1], f32, name="thr_ps", tag="fold_ps")
                nc.tensor.matmul(bp[:], c_ones1x[:], src_col, start=True, stop=True)
                nc.scalar.copy(thr[:], bp[:])

            # C(t0), C(t1) on the 1/4 subsample (band-corrected later)
            bcast_thr(scal[0:1, 0:1])
            f = count_into(0, sub=True)
            nc.scalar.copy(scal[:, 1:2], f[:])
            bcast_thr(scal[0:1, 2:3])
            f = count_into(0, sub=True)
            nc.scalar.copy(scal[:, 3:4], f[:])
            for _rnd in range(SECANT_ROUNDS):
                full = _rnd >= SECANT_ROUNDS - 2
                if _rnd == SECANT_ROUNDS - 2:
                    # switch scale: sub-counts ~ full/4
                    V.tensor_scalar(scal[:, 1:2], scal[:, 1:2], 4.0, 0.0,
                                    Alu.mult, Alu.add)
                    V.tensor_scalar(scal[:, 3:4], scal[:, 3:4], 4.0, 0.0,
                                    Alu.mult, Alu.add)
                # count is monotone non-increasing in t, so sign(dC) = -sign(dT);
                # step = (R - Cb) * dT/dC = (Cb - R) * |dT| / max(|dC|, 1)
                V.tensor_tensor(scal[:, 4:5], scal[:, 3:4], scal[:, 1:2], Alu.subtract)
                V.tensor_scalar(scal[:, 8:9], scal[:, 4:5], -1.0, 0.0, Alu.mult, Alu.add)
                V.tensor_tensor(scal[:, 4:5], scal[:, 4:5], scal[:, 8:9], Alu.max)
                V.tensor_scalar(scal[:, 4:5], scal[:, 4:5], 1.0, 0.0, Alu.max, Alu.add)
                V.tensor_tensor(scal[:, 5:6], scal[:, 2:3], scal[:, 0:1], Alu.subtract)
                V.tensor_scalar(scal[:, 8:9], scal[:, 5:6], -1.0, 0.0, Alu.mult, Alu.add)
                V.tensor_tensor(scal[:, 5:6], scal[:, 5:6], scal[:, 8:9], Alu.max)
                V.reciprocal(scal[:, 8:9], scal[:, 4:5])
                V.tensor_tensor(scal[:, 5:6], scal[:, 5:6], scal[:, 8:9], Alu.mult)
                V.tensor_scalar(scal[:, 6:7], scal[:, 3:4], 1.0,
                                -float(TOPN) if full else -TOPN / 4.0,
                                Alu.mult, Alu.add)
                V.tensor_tensor(scal[:, 6:7], scal[:, 6:7], scal[:, 5:6], Alu.mult)
                V.tensor_copy(scal[:, 0:1], scal[:, 2:3])
                V.tensor_copy(scal[:, 1:2], scal[:, 3:4])
                V.tensor_tensor(scal[:, 2:3], scal[:, 2:3], scal[:, 6:7], Alu.add)
                bcast_thr(scal[0:1, 2:3])
                f = count_into(0, sub=not full)
                nc.scalar.copy(scal[:, 3:4], f[:])
            # C* (full-res count at final thr) is already in acc8 col 0.
            for chn, xt in enumerate(x16):
                V.scalar_tensor_tensor(junk[:], u[:, 0:NW], thr[:], xt[:],
                                       Alu.is_gt, Alu.mult,
                                       accum_out=acc8[:, 1 + chn:2 + chn])
            # band threshold = thr - BAND
            V.tensor_scalar(scal[:, 7:8], scal[:, 2:3], 1.0, -BAND, Alu.mult, Alu.add)
            bcast_thr(scal[0:1, 7:8])
            V.tensor_scalar(junk[:], u[:, 0:NW], thr[:], 0.0, Alu.is_gt, Alu.add,
                            accum_out=acc8[:, 4:5])
            for chn, xt in enumerate(x16):
                V.scalar_tensor_tensor(junk[:], u[:, 0:NW], thr[:], xt[:],
                                       Alu.is_gt, Alu.mult,
                                       accum_out=acc8[:, 5 + chn:6 + chn])
            tps = psml.tile([1, 8], f32, name="tot_ps", tag="fold_ps")
            nc.tensor.matmul(tps[:], c_ones128[:], acc8[:], start=True, stop=True)
            tot = tiny.tile([1, 8], f32, name="tot", tag="tot")
            nc.scalar.copy(tot[:], tps[:])
            # A math: tot = [C*, Sr, Sg, Sb, Cb, Sbr, Sbg, Sbb]
            am = tiny.tile([1, 12], f32, name="am", tag="am")
            # am cols: 0:3 A_img, 3:6 recipA2, 6:9 bias_d (0.5-A), 9 amt, 10 recdc, 11 tmp
            V.tensor_tensor(am[:, 0:3], tot[:, 5:8], tot[:, 1:4], Alu.subtract)  # dS
            V.tensor_tensor(am[:, 11:12], tot[:, 4:5], tot[:, 0:1], Alu.subtract)  # dC
            V.tensor_scalar(am[:, 11:12], am[:, 11:12], 1.0, 0.0, Alu.max, Alu.add)
            V.reciprocal(am[:, 10:11], am[:, 11:12])
            V.tensor_tensor(am[:, 0:3], am[:, 0:3], fbcast(am[:, 10:11], 3), Alu.mult)  # mu
            V.tensor_scalar(am[:, 9:10], tot[:, 0:1], -1.0, float(TOPN), Alu.mult, Alu.add)
            V.tensor_tensor(am[:, 0:3], am[:, 0:3], fbcast(am[:, 9:10], 3), Alu.mult)
            V.tensor_tensor(am[:, 0:3], am[:, 0:3], tot[:, 1:4], Alu.add)  # S + amt*mu
            V.tensor_scalar(am[:, 0:3], am[:, 0:3], 1.0 / TOPN, 0.0, Alu.mult, Alu.add)  # Ax
            V.tensor_scalar(am[:, 3:6], am[:, 0:3], 1.0, 1.0, Alu.mult, Alu.add)  # Ax+1
            V.reciprocal(am[:, 3:6], am[:, 3:6])                      # 1/(Ax+1) = 1/(2A)
            V.tensor_scalar(am[:, 0:3], am[:, 0:3], 0.5, 0.5, Alu.mult, Alu.add)  # A img
            V.tensor_scalar(am[:, 6:9], am[:, 0:3], -1.0, 0.5, Alu.mult, Alu.add)  # .5-A
            # broadcast per-channel scalars to [128,1]
            chsc = tiny.tile([128, 9], f32, name="chsc", tag="chsc")
            for k in range(9):
                bp = psml.tile([128, 1], f32, name="ch_ps", tag="fold_ps")
                nc.tensor.matmul(bp[:], c_ones1x[:], am[0:1, k:k + 1], start=True, stop=True)
                nc.scalar.copy(chsc[:, k:k + 1], bp[:])
            # cols 0:3 A_img, 3:6 recipA2, 6:9 bias_d

            # ---- dark2 (bf16) + p ----
            mxp16 = pp.tile([128, NCHUNK * PADW], bf16, name="mxp16", tag="mxp")
            w116 = pp.tile([128, NCHUNK * PADW], bf16, name="w116", tag="w1")
            memset_pads(mxp16, V)
            yr = srcp.tile([128, NCHUNK * CW], bf16, name="yr", tag="srcp")
            yg = srcp.tile([128, NCHUNK * CW], bf16, name="yg", tag="srcp")
            yb_ = srcp.tile([128, NCHUNK * CW], bf16, name="yb", tag="srcp")
            for yt, xt, k in ((yr, xr, 3), (yg, xg, 4), (yb_, xb, 5)):
                nc.scalar.activation(yt[:], xt[:], Act.Identity,
                                     bias=chsc[:, k:k + 1], scale=chsc[:, k:k + 1])
            V.tensor_tensor(interior(mxp16), cview(yr)[:, :, :],
                            cview(yg)[:, :, :], Alu.min)
            V.tensor_tensor(interior(mxp16), interior(mxp16),
                            cview(yb_)[:, :, :], Alu.min)
            uh16 = pp.tile([128, (NCHUNK + 1) * CW], bf16, name="uh16", tag="uh")
            V.memset(cview(uh16)[:, NCHUNK, :], 1.0)
            hpool(uh16, mxp16, w116, V)
            sh16 = pp.tile([128, NCHUNK * CW], bf16, name="sh16", tag="shv")
            u216 = pp.tile([128, (NCHUNK + 1) * CW], bf16, name="u216", tag="u2")
            V.memset(cview(u216)[:, NCHUNK, :], 1.0)
            vshift_dma(sh16, uh16, 1, nc.sync)
            cmin(u216, uh16, sh16)
            vshift_dma(sh16, u216, 2, nc.sync)
            cmin(uh16, u216, sh16)
            vshift_dma(sh16, uh16, 4, nc.sync)
            cmin(u216, uh16, sh16)
            vshift_dma_down(sh16, u216, 7, nc.sync)
            cmin(uh16, u216, sh16)
            clamp_fix(uh16, u216)
            p = srcp.tile([128, NCHUNK * CW], bf16, name="p", tag="srcp")
            nc.scalar.activation(p[:], uh16[:, 0:NW], Act.Identity, bias=1.0,
                                 scale=-OMEGA)

            # ---- guided filter ----
            Ip = srcp.tile([128, NCHUNK * CW], bf16, name="Ip", tag="srcp")
            V.tensor_tensor(Ip[:], guid[:], p[:], Alu.mult)
            II = srcp.tile([128, NCHUNK * CW], bf16, name="II", tag="srcp")
            nc.scalar.activation(II[:], guid[:], Act.Square)

            cum = pp.tile([128, NCHUNK * CUMW], f32, name="cum", tag="cum")
            cvz = cview(cum, CUMW)
            for c in range(NCHUNK):
                V.memset(cvz[:, c, 0:41], 0.0)

            hbs = {}
            for nm, src_t, ee in (("I", guid, V), ("p", p, G), ("Ip", Ip, V),
                                  ("II", II, G)):
                hb_t = boxes.tile([128, NCHUNK * CW], f32r, name=f"hb{nm}", tag="boxes")
                hbox(hb_t, src_t, cum, V, ee)
                hbs[nm] = hb_t
            means = {}
            for nm, ee in (("I", V), ("p", G), ("Ip", V), ("II", G)):
                mn = boxes.tile([128, NCHUNK * CW], f32, name=f"mean{nm}", tag="boxes")
                vbox(mn, hbs[nm], ee)
                means[nm] = mn
            mI, mp_, mIp, mII = means["I"], means["p"], means["Ip"], means["II"]

            tmp = abt.tile([128, NCHUNK * CW], f32, name="tmp", tag="abt")
            G.tensor_tensor(tmp[:], mI[:], mp_[:], Alu.mult)
            cov = abt.tile([128, NCHUNK * CW], f32, name="cov", tag="abt")
            V.tensor_tensor(cov[:], mIp[:], tmp[:], Alu.subtract)
            sq = abt.tile([128, NCHUNK * CW], f32, name="sq", tag="abt")
            nc.scalar.activation(sq[:], mI[:], Act.Square)
            # var + eps = (mII + EPS) - mI^2 in one pass
            V.scalar_tensor_tensor(sq[:], mII[:], EPS, sq[:], Alu.add, Alu.subtract)
            rec = abt.tile([128, NCHUNK * CW], f32, name="rec", tag="abt")
            V.reciprocal_approx_fast(out=rec[:], in_=sq[:])
            a_t = srcp.tile([128, NCHUNK * CW], f32, name="a_t", tag="srcp")
            V.tensor_tensor(a_t[:], cov[:], rec[:], Alu.mult)
            b_t = srcp.tile([128, NCHUNK * CW], f32, name="b_t", tag="srcp")
            G.tensor_tensor(b_t[:], a_t[:], mI[:], Alu.mult)
            G.tensor_tensor(b_t[:], mp_[:], b_t[:], Alu.subtract)

            hba = boxes.tile([128, NCHUNK * CW], f32r, name="hba", tag="boxes")
            hbox(hba, a_t, cum, V, V)
            hbb = boxes.tile([128, NCHUNK * CW], f32r, name="hbb", tag="boxes")
            hbox(hbb, b_t, cum, V, G)
            mean_a = boxes.tile([128, NCHUNK * CW], f32, name="mean_a", tag="boxes")
            vbox(mean_a, hba, V)
            mean_b = boxes.tile([128, NCHUNK * CW], f32, name="mean_b", tag="boxes")
            vbox(mean_b, hbb, G)

            T_t = abt.tile([128, NCHUNK * CW], f32, name="T_t", tag="abt")
            V.tensor_tensor(T_t[:], mean_a[:], guid[:], Alu.mult)
            V.tensor_tensor(T_t[:], T_t[:], mean_b[:], Alu.add)
            rT = abt.tile([128, NCHUNK * CW], f32, name="rT", tag="abt")
            V.reciprocal_approx_fast(out=rT[:], in_=T_t[:])

            # ---- final: out_c = (0.5 x_c + (0.5 - A_c)) * rT + A_c ----
            for chn, xt in enumerate((xr, xg, xb)):
                d_t = abt.tile([128, NCHUNK * CW], f32, name=f"d{chn}", tag="dout", bufs=2)
                nc.scalar.activation(d_t[:], xt[:], Act.Identity,
                                     bias=chsc[:, 6 + chn:7 + chn], scale=0.5)
                V.tensor_tensor(d_t[:], d_t[:], rT[:], Alu.mult)
                V.tensor_scalar(d_t[:], d_t[:], chsc[:, chn:chn + 1], 0.0,
                                Alu.add, Alu.add)
                nc.scalar.dma_start(out=y_ext[s, chn].rearrange("(c p) w -> p c w",
                                                                p=128),
                                    in_=cview(d_t)[:, :, :])

    nc.compile()
    return nc


def _get_program():
    if "nc" not in _CACHE:
        _CACHE["nc"] = _build()
    return _CACHE["nc"]


def kernel(x: np.ndarray) -> np.ndarray:
    from concourse.bass_utils import run_bass_kernel_spmd
    x = np.ascontiguousarray(np.asarray(x, dtype=np.float32))
    assert x.shape == (16, 3, H, W), x.shape
    nc = _get_program()
    consts = _host_consts()
    in_maps = [{"x": x[2 * i:2 * i + 2], **consts} for i in range(8)]
    res = run_bass_kernel_spmd(nc, in_maps, list(range(8)))
    out = np.concatenate([res.results[i]["y"] for i in range(8)], axis=0)
    return out.astype(np.float32)


# revision 33
# speedup vs baseline: 1.2024x; 1.2024x over previous
"""Dark-Channel-Prior dehazing (DCPGenerator) Trainium2 Bass kernel, v4.

Two samples per core with op-interleaved front-ends (engines execute
in-order, so latency-bound phases of sample 0 — vpool shift DMAs, secant
PE/scalar round-trips — are emitted interleaved with sample 1's
throughput work). Back-ends run sequentially over shared pools.
x is loaded as bf16 via gpsimd SWDGE cast-DMA and reloaded as f32 only
for the output stage; outputs are stored bf16->f32 via SWDGE cast.
"""
import numpy as np
from contextlib import ExitStack

H = 512
W = 512
NCHUNK = 4
CW = 512
NW = NCHUNK * CW
PADW = 526
CUMW = 593          # hbox cum chunk: 41 zeros | 512 cumsum | 40 x cum[511]
WIN_PAD = 7
RADIUS = 40
EPS = 1e-3
OMEGA = 0.95
TOPN = int(0.01 * H * W)          # 2621
T0 = 0.0055
T1 = 0.0085
BAND = 2e-4
SECANT_ROUNDS = 6

_CACHE = {}


def _host_consts():
    n1 = np.minimum(np.arange(H) + RADIUS, H - 1) - np.maximum(np.arange(H) - RADIUS, 0) + 1
    inv_nh = (1.0 / n1).astype(np.float32)
    inv_nw = inv_nh.copy()
    invnh = np.zeros((128, NCHUNK), np.float32)
    for c in range(NCHUNK):
        invnh[:, c] = inv_nh[c * 128:(c + 1) * 128]
    invnw_rep = np.broadcast_to(inv_nw[None, :], (128, W)).copy()
    k = np.arange(128)[:, None]
    p = np.arange(128)[None, :]
    band = (np.abs(k - p) <= RADIUS).astype(np.float32)
    bu = (k >= p + 128 - RADIUS).astype(np.float32) / 81.0 / 81.0
    bd = (k <= p - (128 - RADIUS)).astype(np.float32) / 81.0 / 81.0
    bms = []
    for c in range(NCHUNK):
        bms.append(band * inv_nh[c * 128:(c + 1) * 128][None, :] / 81.0)
    fixl = np.broadcast_to((81.0 * inv_nw[0:RADIUS])[None, :], (128, RADIUS)).copy()
    fixr = np.broadcast_to((81.0 * inv_nw[W - RADIUS:])[None, :], (128, RADIUS)).copy()
    return {"invnw": invnw_rep, "fixl": fixl, "fixr": fixr,
            "bm0": bms[0], "bm1": bms[1], "bm3": bms[3], "bu": bu, "bd": bd}


def _build():
    import concourse.bacc as bacc
    import concourse.tile as tile
    import concourse.bass as bass
    from concourse import mybir

    f32 = mybir.dt.float32
    f32r = mybir.dt.float32r
    bf16 = mybir.dt.bfloat16
    Alu = mybir.AluOpType
    Act = mybir.ActivationFunctionType

    nc = bacc.Bacc("TRN2", target_bir_lowering=False, debug=False, num_devices=8)
    V = nc.vector

    x_ext = nc.dram_tensor("x", [2, 3, H, W], f32, kind="ExternalInput").ap()
    band_exts = {nm: nc.dram_tensor(nm, [128, 128], f32, kind="ExternalInput").ap()
                 for nm in ("bm0", "bm1", "bm3", "bu", "bd")}
    invnw_ext = nc.dram_tensor("invnw", [128, W], f32, kind="ExternalInput").ap()
    fixl_ext = nc.dram_tensor("fixl", [128, RADIUS], f32, kind="ExternalInput").ap()
    fixr_ext = nc.dram_tensor("fixr", [128, RADIUS], f32, kind="ExternalInput").ap()
    y_ext = nc.dram_tensor("y", [2, 3, H, W], f32, kind="ExternalOutput").ap()

    def cview(t, width=CW):
        return t.rearrange("p (c w) -> p c w", w=width)

    def fbcast(ap_col, n):
        return bass.AP(tensor=ap_col.tensor, offset=ap_col.offset,
                       ap=[list(p) for p in ap_col.ap[:-1]] + [[0, n]])

    with ExitStack() as ctx:
        tc = ctx.enter_context(tile.TileContext(nc))

        cpool = ctx.enter_context(tc.tile_pool(name="cpool", bufs=1))
        big = ctx.enter_context(tc.tile_pool(name="big", bufs=2))
        pp = ctx.enter_context(tc.tile_pool(name="pp", bufs=1))
        boxes = ctx.enter_context(tc.tile_pool(name="boxes", bufs=5))
        srcp = ctx.enter_context(tc.tile_pool(name="srcp", bufs=3))
        abt = ctx.enter_context(tc.tile_pool(name="abt", bufs=3))
        tiny = ctx.enter_context(tc.tile_pool(name="tiny", bufs=1))
        pbig = ctx.enter_context(tc.tile_pool(name="pbig", bufs=2, space="PSUM"))
        psml = ctx.enter_context(tc.tile_pool(name="psml", bufs=2, space="PSUM"))

        # ---- constants ----
        c_band = {}
        stage = cpool.tile([128, 128], f32, name="s_band")
        for nm in ("bm0", "bm1", "bm3", "bu", "bd"):
            nc.sync.dma_start(out=stage[:], in_=band_exts[nm][:])
            c_band[nm] = cpool.tile([128, 128], f32r, name=f"c_{nm}")
            nc.scalar.copy(c_band[nm][:], stage[:])
        c_bm = [c_band["bm0"], c_band["bm1"], c_band["bm1"], c_band["bm3"]]
        c_invnw = cpool.tile([128, W], f32, name="c_invnw")
        nc.sync.dma_start(out=c_invnw[:], in_=invnw_ext[:])
        c_fixl = cpool.tile([128, RADIUS], f32, name="c_fixl")
        nc.sync.dma_start(out=c_fixl[:], in_=fixl_ext[:])
        c_fixr = cpool.tile([128, RADIUS], f32, name="c_fixr")
        nc.sync.dma_start(out=c_fixr[:], in_=fixr_ext[:])
        c_ones128 = cpool.tile([128, 1], f32, name="c_ones128")
        V.memset(c_ones128[:], 1.0)
        c_ones1x = cpool.tile([1, 128], f32, name="c_ones1x")
        V.memset(c_ones1x[:], 1.0)
        c_zeros = cpool.tile([128, CW], f32, name="c_zeros")
        V.memset(c_zeros[:], 0.0)
        c_e07 = cpool.tile([128, 7], bf16, name="c_e07")
        V.memset(c_e07[:], 0.0)
        V.memset(c_e07[0:1, :], 1.0)

        # ---------------------------------------------------------- helpers
        def interior(t):
            return cview(t, PADW)[:, :, WIN_PAD:WIN_PAD + CW]

        def memset_pads(t):
            v = cview(t, PADW)
            for c in range(NCHUNK):
                V.memset(v[:, c, 0:WIN_PAD], 1.0)
                V.memset(v[:, c, PADW - WIN_PAD:PADW], 1.0)

        def hpool(dst, padded, w1):
            a = cview(padded, PADW)
            b = cview(w1, PADW)
            d = cview(dst)
            V.tensor_tensor(b[:, :, 0:525], a[:, :, 0:525], a[:, :, 1:526], Alu.min)
            V.tensor_tensor(a[:, :, 0:523], b[:, :, 0:523], b[:, :, 2:525], Alu.min)
            V.tensor_tensor(b[:, :, 0:519], a[:, :, 0:519], a[:, :, 4:523], Alu.min)
            V.tensor_tensor(d[:, 0:NCHUNK, :], b[:, :, 0:512], b[:, :, 7:519],
                            Alu.min)

        def vshift_dma(dst, src, sft):
            dv, sv = cview(dst), cview(src)
            nc.sync.dma_start(out=dv[0:128 - sft, :, :],
                              in_=sv[sft:128, 0:NCHUNK, :])
            nc.sync.dma_start(out=dv[128 - sft:128, :, :],
                              in_=sv[0:sft, 1:NCHUNK + 1, :])

        def vshift_dma_down(dst, src, sft):
            dv, sv = cview(dst), cview(src)
            nc.sync.dma_start(out=dv[sft:128, :, :], in_=sv[0:128 - sft, 0:NCHUNK, :])
            nc.sync.dma_start(out=dv[0:sft, 1:NCHUNK, :],
                              in_=sv[128 - sft:128, 0:NCHUNK - 1, :])

        def cmin(dst_t, a_t2, b_t2):
            V.tensor_tensor(dst_t[:, 0:NW], a_t2[:, 0:NW], b_t2[:, 0:NW], Alu.min)

        def clamp_fix(dst_t, src_t):
            bc = pbig.tile([7, CW], f32, name="clamp_ps", tag="clamp")
            nc.tensor.matmul(bc[:], c_e07[:], cview(src_t)[:, 0, :],
                             start=True, stop=True)
            V.tensor_tensor(cview(dst_t)[0:7, 0, :], cview(src_t)[0:7, 0, :],
                            bc[:], Alu.min)

        def hbox(dst, src, cum):
            sv, dv, cv = cview(src), cview(dst), cview(cum, CUMW)
            for c in range(NCHUNK):
                V.tensor_tensor_scan(cv[:, c, 41:553], sv[:, c, :], c_zeros[:],
                                     0.0, Alu.add, Alu.add)
            for c in range(NCHUNK):
                V.tensor_copy(cv[:, c, 553:593], fbcast(cv[:, c, 552:553], 40))
            V.tensor_tensor(dv[:, :, :], cv[:, :, 81:593], cv[:, :, 0:512],
                            Alu.subtract)

        def vbox(dst, src):
            sv, dv = cview(src), cview(dst)
            for c in range(NCHUNK):
                ops = []
                if c > 0:
                    ops.append((c_band["bu"], c - 1))
                ops.append((c_bm[c], c))
                if c < NCHUNK - 1:
                    ops.append((c_band["bd"], c + 1))
                ps = pbig.tile([128, CW], f32, name="vps", tag="vps")
                for i, (mat, sc_) in enumerate(ops):
                    nc.tensor.matmul(ps[:], mat[:], sv[:, sc_, :],
                                     start=(i == 0), stop=(i == len(ops) - 1))
                nc.scalar.copy(dv[:, c, :], ps[:])
            V.tensor_tensor(dv[:, :, 0:RADIUS], dv[:, :, 0:RADIUS],
                            c_fixl[:].unsqueeze(1).broadcast_to(
                                [128, NCHUNK, RADIUS]), Alu.mult)
            V.tensor_tensor(dv[:, :, CW - RADIUS:CW], dv[:, :, CW - RADIUS:CW],
                            c_fixr[:].unsqueeze(1).broadcast_to(
                                [128, NCHUNK, RADIUS]), Alu.mult)

        # ---------------------------------------------- per-sample frontend
        ST = [dict(), dict()]
        junk = None  # shared count scratch, created lazily (aliases w1)

        def f_load(s):
            st = ST[s]
            st["x16"] = []
            for chn in range(3):
                t16 = srcp.tile([128, NW], bf16, name=f"x16_{s}_{chn}",
                                tag=f"x16_{s}_{chn}", bufs=1)
                nc.gpsimd.dma_start(
                    out=cview(t16)[:, :, :],
                    in_=x_ext[s, chn].rearrange("(c p) w -> p c w", p=128))
                st["x16"].append(t16)

        def f_guid(s):
            st = ST[s]
            xr16, xg16, xb16 = st["x16"]
            Ia = pp.tile([128, NW], bf16, name=f"Ia{s}", tag="ia")
            guid = pp.tile([128, NW], bf16, name=f"guid{s}", tag=f"guid{s}")
            nc.scalar.activation(guid[:], xr16[:], Act.Copy, bias=0.5, scale=0.14945)
            V.scalar_tensor_tensor(Ia[:], xg16[:], 0.2935, guid[:], Alu.mult, Alu.add)
            V.scalar_tensor_tensor(guid[:], xb16[:], 0.057, Ia[:], Alu.mult, Alu.add)
            st["guid"] = guid

        def f_dark_pools(s, second):
            """chan-min + hpool into per-sample uh. second=True uses y tiles."""
            st = ST[s]
            mxp = pp.tile([128, NCHUNK * PADW], bf16, name=f"mxp{s}", tag="mxp")
            w1 = pp.tile([128, NCHUNK * PADW], bf16, name=f"w1{s}", tag="w1")
            memset_pads(mxp)
            if not second:
                a0, a1, a2 = st["x16"]
                V.tensor_tensor(interior(mxp), cview(a0)[:, :, :],
                                cview(a1)[:, :, :], Alu.min)
                V.tensor_tensor(interior(mxp), interior(mxp),
                                cview(a2)[:, :, :], Alu.min)
            else:
                chsc = st["chsc"]
                ytmp = pp.tile([128, NW], bf16, name=f"yt{s}", tag="ytmp")
                nc.scalar.activation(interior(mxp), st["x16"][0][:], Act.Identity,
                                     bias=chsc[:, 3:4], scale=chsc[:, 3:4])
                nc.scalar.activation(ytmp[:], st["x16"][1][:], Act.Identity,
                                     bias=chsc[:, 4:5], scale=chsc[:, 4:5])
                V.tensor_tensor(interior(mxp), interior(mxp),
                                cview(ytmp)[:, :, :], Alu.min)
                nc.scalar.activation(ytmp[:], st["x16"][2][:], Act.Identity,
                                     bias=chsc[:, 5:6], scale=chsc[:, 5:6])
                V.tensor_tensor(interior(mxp), interior(mxp),
                                cview(ytmp)[:, :, :], Alu.min)
            uh = pp.tile([128, (NCHUNK + 1) * CW], bf16, name=f"uh{s}",
                         tag=f"uh{s}")
            V.memset(cview(uh)[:, NCHUNK, :], 1.0)
            hpool(uh, mxp, w1)
            u2 = pp.tile([128, (NCHUNK + 1) * CW], bf16, name=f"u2{s}",
                         tag=f"u2{s}")
            V.memset(cview(u2)[:, NCHUNK, :], 1.0)
            sh = pp.tile([128, NW], bf16, name=f"sh{s}",
                         tag=("ia" if s == 0 else f"sh{s}"))
            st["uh"], st["u2"], st["sh"] = uh, u2, sh

        # vpool steps as small callables so the two samples interleave
        def vp_shift(s, step):
            st = ST[s]
            if step == 0:
                vshift_dma(st["sh"], st["uh"], 1)
            elif step == 1:
                vshift_dma(st["sh"], st["u2"], 2)
            elif step == 2:
                vshift_dma(st["sh"], st["uh"], 4)
            else:
                vshift_dma_down(st["sh"], st["u2"], 7)

        def vp_min(s, step):
            st = ST[s]
            if step == 0:
                cmin(st["u2"], st["uh"], st["sh"])
            elif step == 1:
                cmin(st["uh"], st["u2"], st["sh"])
            elif step == 2:
                cmin(st["u2"], st["uh"], st["sh"])
            else:
                cmin(st["uh"], st["u2"], st["sh"])
                clamp_fix(st["uh"], st["u2"])

        def f_secant_init(s):
            st = ST[s]
            st["acc8"] = tiny.tile([128, 8], f32, name=f"acc8{s}", tag=f"acc8{s}")
            V.memset(st["acc8"][:], 0.0)
            st["thr"] = tiny.tile([128, 1], f32, name=f"thr{s}", tag=f"thr{s}")
            st["scal"] = tiny.tile([1, 16], f32, name=f"scal{s}", tag=f"scal{s}")
            V.memset(st["scal"][:], 0.0)
            V.memset(st["scal"][:, 0:1], T0)
            V.memset(st["scal"][:, 2:3], T1)

        def count_into(s, col, sub=False):
            st = ST[s]
            u, acc8, thr = st["uh"], st["acc8"], st["thr"]
            uv = cview(u)
            if sub:
                V.tensor_scalar(cview(junk)[:, 0:2, 0:256],
                                uv[:, 0:NCHUNK:2, 0:CW:2], thr[:], 0.0,
                                Alu.is_gt, Alu.add,
                                accum_out=acc8[:, col:col + 1])
            else:
                V.tensor_scalar(junk[:], u[:, 0:NW], thr[:], 0.0,
                                Alu.is_gt, Alu.add,
                                accum_out=acc8[:, col:col + 1])
            fps = psml.tile([1, 1], f32, name=f"fold{s}", tag=f"fold{s}")
            nc.tensor.matmul(fps[:], c_ones128[:], acc8[:, col:col + 1],
                             start=True, stop=True)
            return fps

        def bcast_thr(s, src_col):
            st = ST[s]
            bp = psml.tile([128, 1], f32, name=f"thrps{s}", tag=f"fold{s}")
            nc.tensor.matmul(bp[:], c_ones1x[:], src_col, start=True, stop=True)
            nc.scalar.copy(st["thr"][:], bp[:])

        def f_count0(s, which):
            scal = ST[s]["scal"]
            col = 0 if which == 0 else 2
            bcast_thr(s, scal[0:1, col:col + 1])
            f = count_into(s, 0, sub=True)
            nc.scalar.copy(scal[:, col + 1:col + 2], f[:])

        def f_secant_round(s, rnd):
            scal = ST[s]["scal"]
            full = rnd >= SECANT_ROUNDS - 2
            if rnd == SECANT_ROUNDS - 2:
                V.tensor_scalar(scal[:, 1:2], scal[:, 1:2], 4.0, 0.0,
                                Alu.mult, Alu.add)
                V.tensor_scalar(scal[:, 3:4], scal[:, 3:4], 4.0, 0.0,
                                Alu.mult, Alu.add)
            V.tensor_tensor(scal[:, 4:5], scal[:, 3:4], scal[:, 1:2], Alu.subtract)
            V.tensor_scalar(scal[:, 8:9], scal[:, 4:5], -1.0, 0.0, Alu.mult, Alu.add)
            V.tensor_tensor(scal[:, 4:5], scal[:, 4:5], scal[:, 8:9], Alu.max)
            V.tensor_scalar(scal[:, 4:5], scal[:, 4:5], 1.0, 0.0, Alu.max, Alu.add)
            V.tensor_tensor(scal[:, 5:6], scal[:, 2:3], scal[:, 0:1], Alu.subtract)
            V.tensor_scalar(scal[:, 8:9], scal[:, 5:6], -1.0, 0.0, Alu.mult, Alu.add)
            V.tensor_tensor(scal[:, 5:6], scal[:, 5:6], scal[:, 8:9], Alu.max)
            V.reciprocal(scal[:, 8:9], scal[:, 4:5])
            V.tensor_tensor(scal[:, 5:6], scal[:, 5:6], scal[:, 8:9], Alu.mult)
            V.tensor_scalar(scal[:, 6:7], scal[:, 3:4], 1.0,
                            -float(TOPN) if full else -TOPN / 4.0,
                            Alu.mult, Alu.add)
            V.tensor_tensor(scal[:, 6:7], scal[:, 6:7], scal[:, 5:6], Alu.mult)
            V.tensor_copy(scal[:, 0:1], scal[:, 2:3])
            V.tensor_copy(scal[:, 1:2], scal[:, 3:4])
            V.tensor_tensor(scal[:, 2:3], scal[:, 2:3], scal[:, 6:7], Alu.add)
            bcast_thr(s, scal[0:1, 2:3])
            f = count_into(s, 0, sub=not full)
            nc.scalar.copy(scal[:, 3:4], f[:])

        def f_msums(s):
            st = ST[s]
            u, acc8, thr = st["uh"], st["acc8"], st["thr"]
            for chn, xt in enumerate(st["x16"]):
                V.scalar_tensor_tensor(junk[:], u[:, 0:NW], thr[:], xt[:],
                                       Alu.is_gt, Alu.mult,
                                       accum_out=acc8[:, 1 + chn:2 + chn])

        def f_bandprep(s):
            st = ST[s]
            scal = st["scal"]
            V.tensor_scalar(scal[:, 7:8], scal[:, 2:3], 1.0, -BAND,
                            Alu.mult, Alu.add)
            bcast_thr(s, scal[0:1, 7:8])

        def f_bandsums(s):
            st = ST[s]
            u, acc8, thr = st["uh"], st["acc8"], st["thr"]
            V.tensor_scalar(junk[:], u[:, 0:NW], thr[:], 0.0, Alu.is_gt,
                            Alu.add, accum_out=acc8[:, 4:5])
            for chn, xt in enumerate(st["x16"]):
                V.scalar_tensor_tensor(junk[:], u[:, 0:NW], thr[:], xt[:],
                                       Alu.is_gt, Alu.mult,
                                       accum_out=acc8[:, 5 + chn:6 + chn])

        def f_afold(s):
            st = ST[s]
            tps = psml.tile([1, 8], f32, name=f"totps{s}", tag=f"fold{s}")
            nc.tensor.matmul(tps[:], c_ones128[:], st["acc8"][:],
                             start=True, stop=True)
            tot = tiny.tile([1, 8], f32, name=f"tot{s}", tag=f"tot{s}")
            nc.scalar.copy(tot[:], tps[:])
            st["tot"] = tot

        def f_amath(s):
            st = ST[s]
            tot = st["tot"]
            am = tiny.tile([1, 12], f32, name=f"am{s}", tag=f"am{s}")
            V.tensor_tensor(am[:, 0:3], tot[:, 5:8], tot[:, 1:4], Alu.subtract)
            V.tensor_tensor(am[:, 11:12], tot[:, 4:5], tot[:, 0:1], Alu.subtract)
            V.tensor_scalar(am[:, 11:12], am[:, 11:12], 1.0, 0.0, Alu.max, Alu.add)
            V.reciprocal(am[:, 10:11], am[:, 11:12])
            V.tensor_tensor(am[:, 0:3], am[:, 0:3], fbcast(am[:, 10:11], 3), Alu.mult)
            V.tensor_scalar(am[:, 9:10], tot[:, 0:1], -1.0, float(TOPN),
                            Alu.mult, Alu.add)
            V.tensor_tensor(am[:, 0:3], am[:, 0:3], fbcast(am[:, 9:10], 3), Alu.mult)
            V.tensor_tensor(am[:, 0:3], am[:, 0:3], tot[:, 1:4], Alu.add)
            V.tensor_scalar(am[:, 0:3], am[:, 0:3], 1.0 / TOPN, 0.0, Alu.mult, Alu.add)
            V.tensor_scalar(am[:, 3:6], am[:, 0:3], 1.0, 1.0, Alu.mult, Alu.add)
            V.reciprocal(am[:, 3:6], am[:, 3:6])
            V.tensor_scalar(am[:, 0:3], am[:, 0:3], 0.5, 0.5, Alu.mult, Alu.add)
            V.tensor_scalar(am[:, 6:9], am[:, 0:3], -1.0, 0.5, Alu.mult, Alu.add)
            st["am"] = am

        def f_chsc(s, k):
            st = ST[s]
            if "chsc" not in st:
                st["chsc"] = tiny.tile([128, 9], f32, name=f"chsc{s}",
                                       tag=f"chsc{s}")
            bp = psml.tile([128, 1], f32, name=f"chps{s}", tag=f"fold{s}")
            nc.tensor.matmul(bp[:], c_ones1x[:], st["am"][0:1, k:k + 1],
                             start=True, stop=True)
            nc.scalar.copy(st["chsc"][:, k:k + 1], bp[:])

        def f_p(s):
            st = ST[s]
            p = pp.tile([128, NW], bf16, name=f"p{s}", tag=f"p{s}")
            nc.scalar.activation(p[:], st["uh"][:, 0:NW], Act.Identity,
                                 bias=1.0, scale=-OMEGA)
            st["p"] = p

        # ---------------------------------------------------------- backend
        def backend(s):
            st = ST[s]
            guid, p, chsc = st["guid"], st["p"], st["chsc"]
            # reload f32 x for the output stage (ready by the time it's used)
            xrld = []
            for chn in range(3):
                t = big.tile([128, NW], f32, name=f"xr{s}_{chn}", tag="xrld")
                nc.scalar.dma_start(out=cview(t)[:, :, :],
                                    in_=x_ext[s, chn].rearrange(
                                        "(c p) w -> p c w", p=128))
                xrld.append(t)

            Ip = srcp.tile([128, NW], bf16, name="Ip", tag="srcp")
            V.tensor_tensor(Ip[:], guid[:], p[:], Alu.mult)
            II = srcp.tile([128, NW], bf16, name="II", tag="srcp")
            nc.scalar.activation(II[:], guid[:], Act.Square)

            cum = pp.tile([128, NCHUNK * CUMW], f32, name="cum", tag="cum")
            cvz = cview(cum, CUMW)
            for c in range(NCHUNK):
                V.memset(cvz[:, c, 0:41], 0.0)

            hbs = {}
            for nm, src_t in (("I", guid), ("p", p), ("Ip", Ip), ("II", II)):
                hb_t = boxes.tile([128, NW], f32r, name=f"hb{nm}", tag="boxes")
                hbox(hb_t, src_t, cum)
                hbs[nm] = hb_t
            means = {}
            for nm in ("I", "p", "Ip", "II"):
                mn = boxes.tile([128, NW], f32, name=f"mean{nm}", tag="boxes")
                vbox(mn, hbs[nm])
                means[nm] = mn
            mI, mp_, mIp, mII = means["I"], means["p"], means["Ip"], means["II"]

            tmp = abt.tile([128, NW], f32, name="tmp", tag="abt")
            V.tensor_tensor(tmp[:], mI[:], mp_[:], Alu.mult)
            cov = abt.tile([128, NW], f32, name="cov", tag="abt")
            V.tensor_tensor(cov[:], mIp[:], tmp[:], Alu.subtract)
            sq = abt.tile([128, NW], f32, name="sq", tag="abt")
            nc.scalar.activation(sq[:], mI[:], Act.Square)
            V.scalar_tensor_tensor(sq[:], mII[:], EPS, sq[:], Alu.add, Alu.subtract)
            rec = abt.tile([128, NW], f32, name="rec", tag="abt")
            V.reciprocal_approx_fast(out=rec[:], in_=sq[:])
            a_t = srcp.tile([128, NW], bf16, name="a_t", tag="srcp")
            V.tensor_tensor(a_t[:], cov[:], rec[:], Alu.mult)
            b_t = srcp.tile([128, NW], bf16, name="b_t", tag="srcp")
            V.tensor_tensor(b_t[:], a_t[:], mI[:], Alu.mult)
            V.tensor_tensor(b_t[:], mp_[:], b_t[:], Alu.subtract)

            hba = boxes.tile([128, NW], f32r, name="hba", tag="boxes")
            hbox(hba, a_t, cum)
            hbb = boxes.tile([128, NW], f32r, name="hbb", tag="boxes")
            hbox(hbb, b_t, cum)
            mean_a = boxes.tile([128, NW], f32, name="mean_a", tag="boxes")
            vbox(mean_a, hba)
            mean_b = boxes.tile([128, NW], f32, name="mean_b", tag="boxes")
            vbox(mean_b, hbb)

            T_t = abt.tile([128, NW], f32, name="T_t", tag="abt")
            V.tensor_tensor(T_t[:], mean_a[:], guid[:], Alu.mult)
            V.tensor_tensor(T_t[:], T_t[:], mean_b[:], Alu.add)
            rT = abt.tile([128, NW], f32, name="rT", tag="abt")
            V.reciprocal_approx_fast(out=rT[:], in_=T_t[:])

            for chn in range(3):
                d_t = abt.tile([128, NW], bf16, name=f"d{chn}", tag="dout", bufs=2)
                nc.scalar.activation(d_t[:], xrld[chn][:], Act.Identity,
                                     bias=chsc[:, 6 + chn:7 + chn], scale=0.5)
                V.tensor_tensor(d_t[:], d_t[:], rT[:], Alu.mult)
                V.tensor_scalar(d_t[:], d_t[:], chsc[:, chn:chn + 1], 0.0,
                                Alu.add, Alu.add)
                nc.gpsimd.dma_start(out=y_ext[s, chn].rearrange(
                                        "(c p) w -> p c w", p=128),
                                    in_=cview(d_t)[:, :, :])

        # ================================================== emission order
        f_load(0)
        f_load(1)
        f_guid(0)
        f_guid(1)
        f_dark_pools(0, second=False)
        f_dark_pools(1, second=False)
        for step in range(4):
            vp_shift(0, step)
            vp_shift(1, step)
            vp_min(0, step)
            vp_min(1, step)
        f_secant_init(0)
        f_secant_init(1)
        junk = pp.tile([128, NW], bf16, name="junk", tag="w1")
        for which in (0, 1):
            f_count0(0, which)
            f_count0(1, which)
        for rnd in range(SECANT_ROUNDS):
            f_secant_round(0, rnd)
            f_secant_round(1, rnd)
        f_msums(0)
        f_msums(1)
        f_bandprep(0)
        f_bandprep(1)
        f_bandsums(0)
        f_bandsums(1)
        f_afold(0)
        f_afold(1)
        f_amath(0)
        f_amath(1)
        for k in range(9):
            f_chsc(0, k)
            f_chsc(1, k)
        f_dark_pools(0, second=True)
        f_dark_pools(1, second=True)
        for step in range(4):
            vp_shift(0, step)
            vp_shift(1, step)
            vp_min(0, step)
            vp_min(1, step)
        f_p(0)
        f_p(1)
        backend(0)
        backend(1)

    nc.compile()
    return nc


def _get_program():
    if "nc" not in _CACHE:
        _CACHE["nc"] = _build()
    return _CACHE["nc"]


def kernel(x: np.ndarray) -> np.ndarray:
    from concourse.bass_utils import run_bass_kernel_spmd
    x = np.ascontiguousarray(np.asarray(x, dtype=np.float32))
    assert x.shape == (16, 3, H, W), x.shape
    nc = _get_program()
    consts = _host_consts()
    in_maps = [{"x": x[2 * i:2 * i + 2], **consts} for i in range(8)]
    res = run_bass_kernel_spmd(nc, in_maps, list(range(8)))
    out = np.concatenate([res.results[i]["y"] for i in range(8)], axis=0)
    return out.astype(np.float32)


# revision 34
# speedup vs baseline: 1.2095x; 1.0059x over previous
"""Dark-Channel-Prior dehazing (DCPGenerator) Trainium2 Bass kernel, v4.

Two samples per core with op-interleaved front-ends (engines execute
in-order, so latency-bound phases of sample 0 — vpool shift DMAs, secant
PE/scalar round-trips — are emitted interleaved with sample 1's
throughput work). Back-ends run sequentially over shared pools.
x is loaded as bf16 via gpsimd SWDGE cast-DMA and reloaded as f32 only
for the output stage; outputs are stored bf16->f32 via SWDGE cast.
"""
import numpy as np
from contextlib import ExitStack

H = 512
W = 512
NCHUNK = 4
CW = 512
NW = NCHUNK * CW
PADW = 526
CUMW = 593          # hbox cum chunk: 41 zeros | 512 cumsum | 40 x cum[511]
WIN_PAD = 7
RADIUS = 40
EPS = 1e-3
OMEGA = 0.95
TOPN = int(0.01 * H * W)          # 2621
T0 = 0.0055
T1 = 0.0085
BAND = 2e-4
SECANT_ROUNDS = 6

_CACHE = {}


def _host_consts():
    n1 = np.minimum(np.arange(H) + RADIUS, H - 1) - np.maximum(np.arange(H) - RADIUS, 0) + 1
    inv_nh = (1.0 / n1).astype(np.float32)
    inv_nw = inv_nh.copy()
    invnh = np.zeros((128, NCHUNK), np.float32)
    for c in range(NCHUNK):
        invnh[:, c] = inv_nh[c * 128:(c + 1) * 128]
    invnw_rep = np.broadcast_to(inv_nw[None, :], (128, W)).copy()
    k = np.arange(128)[:, None]
    p = np.arange(128)[None, :]
    band = (np.abs(k - p) <= RADIUS).astype(np.float32)
    bu = (k >= p + 128 - RADIUS).astype(np.float32) / 81.0 / 81.0
    bd = (k <= p - (128 - RADIUS)).astype(np.float32) / 81.0 / 81.0
    bms = []
    for c in range(NCHUNK):
        bms.append(band * inv_nh[c * 128:(c + 1) * 128][None, :] / 81.0)
    fixl = np.broadcast_to((81.0 * inv_nw[0:RADIUS])[None, :], (128, RADIUS)).copy()
    fixr = np.broadcast_to((81.0 * inv_nw[W - RADIUS:])[None, :], (128, RADIUS)).copy()
    return {"invnw": invnw_rep, "fixl": fixl, "fixr": fixr,
            "bm0": bms[0], "bm1": bms[1], "bm3": bms[3], "bu": bu, "bd": bd}


def _build():
    import concourse.bacc as bacc
    import concourse.tile as tile
    import concourse.bass as bass
    from concourse import mybir

    f32 = mybir.dt.float32
    f32r = mybir.dt.float32r
    bf16 = mybir.dt.bfloat16
    Alu = mybir.AluOpType
    Act = mybir.ActivationFunctionType

    nc = bacc.Bacc("TRN2", target_bir_lowering=False, debug=False, num_devices=8)
    V = nc.vector

    x_ext = nc.dram_tensor("x", [2, 3, H, W], f32, kind="ExternalInput").ap()
    band_exts = {nm: nc.dram_tensor(nm, [128, 128], f32, kind="ExternalInput").ap()
                 for nm in ("bm0", "bm1", "bm3", "bu", "bd")}
    invnw_ext = nc.dram_tensor("invnw", [128, W], f32, kind="ExternalInput").ap()
    fixl_ext = nc.dram_tensor("fixl", [128, RADIUS], f32, kind="ExternalInput").ap()
    fixr_ext = nc.dram_tensor("fixr", [128, RADIUS], f32, kind="ExternalInput").ap()
    y_ext = nc.dram_tensor("y", [2, 3, H, W], f32, kind="ExternalOutput").ap()

    def cview(t, width=CW):
        return t.rearrange("p (c w) -> p c w", w=width)

    def fbcast(ap_col, n):
        return bass.AP(tensor=ap_col.tensor, offset=ap_col.offset,
                       ap=[list(p) for p in ap_col.ap[:-1]] + [[0, n]])

    with ExitStack() as ctx:
        tc = ctx.enter_context(tile.TileContext(nc))

        cpool = ctx.enter_context(tc.tile_pool(name="cpool", bufs=1))
        big = ctx.enter_context(tc.tile_pool(name="big", bufs=2))
        pp = ctx.enter_context(tc.tile_pool(name="pp", bufs=1))
        boxes = ctx.enter_context(tc.tile_pool(name="boxes", bufs=5))
        srcp = ctx.enter_context(tc.tile_pool(name="srcp", bufs=3))
        abt = ctx.enter_context(tc.tile_pool(name="abt", bufs=3))
        tiny = ctx.enter_context(tc.tile_pool(name="tiny", bufs=1))
        pbig = ctx.enter_context(tc.tile_pool(name="pbig", bufs=2, space="PSUM"))
        psml = ctx.enter_context(tc.tile_pool(name="psml", bufs=2, space="PSUM"))

        # ---- constants ----
        c_band = {}
        stage = cpool.tile([128, 128], f32, name="s_band")
        for nm in ("bm0", "bm1", "bm3", "bu", "bd"):
            nc.sync.dma_start(out=stage[:], in_=band_exts[nm][:])
            c_band[nm] = cpool.tile([128, 128], f32r, name=f"c_{nm}")
            nc.scalar.copy(c_band[nm][:], stage[:])
        c_bm = [c_band["bm0"], c_band["bm1"], c_band["bm1"], c_band["bm3"]]
        c_invnw = cpool.tile([128, W], f32, name="c_invnw")
        nc.sync.dma_start(out=c_invnw[:], in_=invnw_ext[:])
        c_fixl = cpool.tile([128, RADIUS], f32, name="c_fixl")
        nc.sync.dma_start(out=c_fixl[:], in_=fixl_ext[:])
        c_fixr = cpool.tile([128, RADIUS], f32, name="c_fixr")
        nc.sync.dma_start(out=c_fixr[:], in_=fixr_ext[:])
        c_ones128 = cpool.tile([128, 1], f32, name="c_ones128")
        V.memset(c_ones128[:], 1.0)
        c_ones1x = cpool.tile([1, 128], f32, name="c_ones1x")
        V.memset(c_ones1x[:], 1.0)
        c_zeros = cpool.tile([128, CW], f32, name="c_zeros")
        V.memset(c_zeros[:], 0.0)
        c_e07 = cpool.tile([128, 7], bf16, name="c_e07")
        V.memset(c_e07[:], 0.0)
        V.memset(c_e07[0:1, :], 1.0)

        # ---------------------------------------------------------- helpers
        def interior(t):
            return cview(t, PADW)[:, :, WIN_PAD:WIN_PAD + CW]

        def memset_pads(t):
            v = cview(t, PADW)
            for c in range(NCHUNK):
                V.memset(v[:, c, 0:WIN_PAD], 1.0)
                V.memset(v[:, c, PADW - WIN_PAD:PADW], 1.0)

        def hpool(dst, padded, w1):
            a = cview(padded, PADW)
            b = cview(w1, PADW)
            d = cview(dst)
            V.tensor_tensor(b[:, :, 0:525], a[:, :, 0:525], a[:, :, 1:526], Alu.min)
            V.tensor_tensor(a[:, :, 0:523], b[:, :, 0:523], b[:, :, 2:525], Alu.min)
            V.tensor_tensor(b[:, :, 0:519], a[:, :, 0:519], a[:, :, 4:523], Alu.min)
            V.tensor_tensor(d[:, 0:NCHUNK, :], b[:, :, 0:512], b[:, :, 7:519],
                            Alu.min)

        def vshift_dma(dst, src, sft):
            dv, sv = cview(dst), cview(src)
            nc.sync.dma_start(out=dv[0:128 - sft, :, :],
                              in_=sv[sft:128, 0:NCHUNK, :])
            nc.sync.dma_start(out=dv[128 - sft:128, :, :],
                              in_=sv[0:sft, 1:NCHUNK + 1, :])

        def vshift_dma_down(dst, src, sft):
            dv, sv = cview(dst), cview(src)
            nc.sync.dma_start(out=dv[sft:128, :, :], in_=sv[0:128 - sft, 0:NCHUNK, :])
            nc.sync.dma_start(out=dv[0:sft, 1:NCHUNK, :],
                              in_=sv[128 - sft:128, 0:NCHUNK - 1, :])

        def cmin(dst_t, a_t2, b_t2):
            V.tensor_tensor(dst_t[:, 0:NW], a_t2[:, 0:NW], b_t2[:, 0:NW], Alu.min)

        def clamp_fix(dst_t, src_t):
            bc = pbig.tile([7, CW], f32, name="clamp_ps", tag="clamp")
            nc.tensor.matmul(bc[:], c_e07[:], cview(src_t)[:, 0, :],
                             start=True, stop=True)
            V.tensor_tensor(cview(dst_t)[0:7, 0, :], cview(src_t)[0:7, 0, :],
                            bc[:], Alu.min)

        def hbox(dst, src, cum):
            sv, dv, cv = cview(src), cview(dst), cview(cum, CUMW)
            for c in range(NCHUNK):
                V.tensor_tensor_scan(cv[:, c, 41:553], sv[:, c, :], c_zeros[:],
                                     0.0, Alu.add, Alu.add)
            for c in range(NCHUNK):
                V.tensor_copy(cv[:, c, 553:593], fbcast(cv[:, c, 552:553], 40))
            V.tensor_tensor(dv[:, :, :], cv[:, :, 81:593], cv[:, :, 0:512],
                            Alu.subtract)

        def vbox(dst, src):
            sv, dv = cview(src), cview(dst)
            for c in range(NCHUNK):
                ops = []
                if c > 0:
                    ops.append((c_band["bu"], c - 1))
                ops.append((c_bm[c], c))
                if c < NCHUNK - 1:
                    ops.append((c_band["bd"], c + 1))
                ps = pbig.tile([128, CW], f32, name="vps", tag="vps")
                for i, (mat, sc_) in enumerate(ops):
                    nc.tensor.matmul(ps[:], mat[:], sv[:, sc_, :],
                                     start=(i == 0), stop=(i == len(ops) - 1))
                nc.scalar.copy(dv[:, c, :], ps[:])
            V.tensor_tensor(dv[:, :, 0:RADIUS], dv[:, :, 0:RADIUS],
                            c_fixl[:].unsqueeze(1).broadcast_to(
                                [128, NCHUNK, RADIUS]), Alu.mult)
            V.tensor_tensor(dv[:, :, CW - RADIUS:CW], dv[:, :, CW - RADIUS:CW],
                            c_fixr[:].unsqueeze(1).broadcast_to(
                                [128, NCHUNK, RADIUS]), Alu.mult)

        # ---------------------------------------------- per-sample frontend
        ST = [dict(), dict()]
        junk = None  # shared count scratch, created lazily (aliases w1)

        def f_load(s):
            st = ST[s]
            st["x16"] = []
            for chn in range(3):
                t16 = srcp.tile([128, NW], bf16, name=f"x16_{s}_{chn}",
                                tag=f"x16_{s}_{chn}", bufs=1)
                nc.gpsimd.dma_start(
                    out=cview(t16)[:, :, :],
                    in_=x_ext[s, chn].rearrange("(c p) w -> p c w", p=128))
                st["x16"].append(t16)

        def f_guid(s):
            st = ST[s]
            xr16, xg16, xb16 = st["x16"]
            Ia = pp.tile([128, NW], bf16, name=f"Ia{s}", tag="ia")
            guid = pp.tile([128, NW], bf16, name=f"guid{s}", tag=f"guid{s}")
            nc.scalar.activation(guid[:], xr16[:], Act.Copy, bias=0.5, scale=0.14945)
            V.scalar_tensor_tensor(Ia[:], xg16[:], 0.2935, guid[:], Alu.mult, Alu.add)
            V.scalar_tensor_tensor(guid[:], xb16[:], 0.057, Ia[:], Alu.mult, Alu.add)
            st["guid"] = guid

        def f_dark_pools(s, second):
            """chan-min + hpool into per-sample uh. second=True uses y tiles."""
            st = ST[s]
            mxp = pp.tile([128, NCHUNK * PADW], bf16, name=f"mxp{s}", tag="mxp")
            w1 = pp.tile([128, NCHUNK * PADW], bf16, name=f"w1{s}", tag="w1")
            memset_pads(mxp)
            if not second:
                a0, a1, a2 = st["x16"]
                V.tensor_tensor(interior(mxp), cview(a0)[:, :, :],
                                cview(a1)[:, :, :], Alu.min)
                V.tensor_tensor(interior(mxp), interior(mxp),
                                cview(a2)[:, :, :], Alu.min)
            else:
                chsc = st["chsc"]
                ytmp = pp.tile([128, NW], bf16, name=f"yt{s}", tag="ytmp")
                nc.scalar.activation(interior(mxp), st["x16"][0][:], Act.Identity,
                                     bias=chsc[:, 3:4], scale=chsc[:, 3:4])
                nc.scalar.activation(ytmp[:], st["x16"][1][:], Act.Identity,
                                     bias=chsc[:, 4:5], scale=chsc[:, 4:5])
                V.tensor_tensor(interior(mxp), interior(mxp),
                                cview(ytmp)[:, :, :], Alu.min)
                nc.scalar.activation(ytmp[:], st["x16"][2][:], Act.Identity,
                                     bias=chsc[:, 5:6], scale=chsc[:, 5:6])
                V.tensor_tensor(interior(mxp), interior(mxp),
                                cview(ytmp)[:, :, :], Alu.min)
            uh = pp.tile([128, (NCHUNK + 1) * CW], bf16, name=f"uh{s}",
                         tag=f"uh{s}")
            V.memset(cview(uh)[:, NCHUNK, :], 1.0)
            hpool(uh, mxp, w1)
            u2 = pp.tile([128, (NCHUNK + 1) * CW], bf16, name=f"u2{s}",
                         tag=f"u2{s}")
            V.memset(cview(u2)[:, NCHUNK, :], 1.0)
            sh = pp.tile([128, NW], bf16, name=f"sh{s}",
                         tag=("ia" if s == 0 else f"sh{s}"))
            st["uh"], st["u2"], st["sh"] = uh, u2, sh

        # vpool steps as small callables so the two samples interleave
        def vp_shift(s, step):
            st = ST[s]
            if step == 0:
                vshift_dma(st["sh"], st["uh"], 1)
            elif step == 1:
                vshift_dma(st["sh"], st["u2"], 2)
            elif step == 2:
                vshift_dma(st["sh"], st["uh"], 4)
            else:
                vshift_dma_down(st["sh"], st["u2"], 7)

        def vp_min(s, step):
            st = ST[s]
            if step == 0:
                cmin(st["u2"], st["uh"], st["sh"])
            elif step == 1:
                cmin(st["uh"], st["u2"], st["sh"])
            elif step == 2:
                cmin(st["u2"], st["uh"], st["sh"])
            else:
                cmin(st["uh"], st["u2"], st["sh"])
                clamp_fix(st["uh"], st["u2"])

        def f_secant_init(s):
            st = ST[s]
            st["acc8"] = tiny.tile([128, 8], f32, name=f"acc8{s}", tag=f"acc8{s}")
            V.memset(st["acc8"][:], 0.0)
            st["thr"] = tiny.tile([128, 1], f32, name=f"thr{s}", tag=f"thr{s}")
            st["scal"] = tiny.tile([1, 16], f32, name=f"scal{s}", tag=f"scal{s}")
            V.memset(st["scal"][:], 0.0)
            V.memset(st["scal"][:, 0:1], T0)
            V.memset(st["scal"][:, 2:3], T1)

        def count_into(s, col, sub=False):
            st = ST[s]
            u, acc8, thr = st["uh"], st["acc8"], st["thr"]
            uv = cview(u)
            if sub:
                V.tensor_scalar(cview(junk)[:, 0:2, 0:256],
                                uv[:, 0:NCHUNK:2, 0:CW:2], thr[:], 0.0,
                                Alu.is_gt, Alu.add,
                                accum_out=acc8[:, col:col + 1])
            else:
                V.tensor_scalar(junk[:], u[:, 0:NW], thr[:], 0.0,
                                Alu.is_gt, Alu.add,
                                accum_out=acc8[:, col:col + 1])
            fps = psml.tile([1, 1], f32, name=f"fold{s}", tag=f"fold{s}")
            nc.tensor.matmul(fps[:], c_ones128[:], acc8[:, col:col + 1],
                             start=True, stop=True)
            return fps

        def bcast_thr(s, src_col):
            st = ST[s]
            bp = psml.tile([128, 1], f32, name=f"thrps{s}", tag=f"fold{s}")
            nc.tensor.matmul(bp[:], c_ones1x[:], src_col, start=True, stop=True)
            nc.scalar.copy(st["thr"][:], bp[:])

        def f_count0(s, which):
            scal = ST[s]["scal"]
            col = 0 if which == 0 else 2
            bcast_thr(s, scal[0:1, col:col + 1])
            f = count_into(s, 0, sub=True)
            nc.scalar.copy(scal[:, col + 1:col + 2], f[:])

        def f_secant_round(s, rnd):
            scal = ST[s]["scal"]
            full = rnd >= SECANT_ROUNDS - 2
            if rnd == SECANT_ROUNDS - 2:
                V.tensor_scalar(scal[:, 1:2], scal[:, 1:2], 4.0, 0.0,
                                Alu.mult, Alu.add)
                V.tensor_scalar(scal[:, 3:4], scal[:, 3:4], 4.0, 0.0,
                                Alu.mult, Alu.add)
            V.tensor_tensor(scal[:, 4:5], scal[:, 3:4], scal[:, 1:2], Alu.subtract)
            V.tensor_scalar(scal[:, 8:9], scal[:, 4:5], -1.0, 0.0, Alu.mult, Alu.add)
            V.tensor_tensor(scal[:, 4:5], scal[:, 4:5], scal[:, 8:9], Alu.max)
            V.tensor_scalar(scal[:, 4:5], scal[:, 4:5], 1.0, 0.0, Alu.max, Alu.add)
            V.tensor_tensor(scal[:, 5:6], scal[:, 2:3], scal[:, 0:1], Alu.subtract)
            V.tensor_scalar(scal[:, 8:9], scal[:, 5:6], -1.0, 0.0, Alu.mult, Alu.add)
            V.tensor_tensor(scal[:, 5:6], scal[:, 5:6], scal[:, 8:9], Alu.max)
            V.reciprocal(scal[:, 8:9], scal[:, 4:5])
            V.tensor_tensor(scal[:, 5:6], scal[:, 5:6], scal[:, 8:9], Alu.mult)
            V.tensor_scalar(scal[:, 6:7], scal[:, 3:4], 1.0,
                            -float(TOPN) if full else -TOPN / 4.0,
                            Alu.mult, Alu.add)
            V.tensor_tensor(scal[:, 6:7], scal[:, 6:7], scal[:, 5:6], Alu.mult)
            V.tensor_copy(scal[:, 0:1], scal[:, 2:3])
            V.tensor_copy(scal[:, 1:2], scal[:, 3:4])
            V.tensor_tensor(scal[:, 2:3], scal[:, 2:3], scal[:, 6:7], Alu.add)
            bcast_thr(s, scal[0:1, 2:3])
            f = count_into(s, 0, sub=not full)
            nc.scalar.copy(scal[:, 3:4], f[:])

        def f_msums(s):
            st = ST[s]
            u, acc8, thr = st["uh"], st["acc8"], st["thr"]
            for chn, xt in enumerate(st["x16"]):
                V.scalar_tensor_tensor(junk[:], u[:, 0:NW], thr[:], xt[:],
                                       Alu.is_gt, Alu.mult,
                                       accum_out=acc8[:, 1 + chn:2 + chn])

        def f_bandprep(s):
            st = ST[s]
            scal = st["scal"]
            V.tensor_scalar(scal[:, 7:8], scal[:, 2:3], 1.0, -BAND,
                            Alu.mult, Alu.add)
            bcast_thr(s, scal[0:1, 7:8])

        def f_bandsums(s):
            st = ST[s]
            u, acc8, thr = st["uh"], st["acc8"], st["thr"]
            V.tensor_scalar(junk[:], u[:, 0:NW], thr[:], 0.0, Alu.is_gt,
                            Alu.add, accum_out=acc8[:, 4:5])
            for chn, xt in enumerate(st["x16"]):
                V.scalar_tensor_tensor(junk[:], u[:, 0:NW], thr[:], xt[:],
                                       Alu.is_gt, Alu.mult,
                                       accum_out=acc8[:, 5 + chn:6 + chn])

        def f_afold(s):
            st = ST[s]
            tps = psml.tile([1, 8], f32, name=f"totps{s}", tag=f"fold{s}")
            nc.tensor.matmul(tps[:], c_ones128[:], st["acc8"][:],
                             start=True, stop=True)
            tot = tiny.tile([1, 8], f32, name=f"tot{s}", tag=f"tot{s}")
            nc.scalar.copy(tot[:], tps[:])
            st["tot"] = tot

        def f_amath(s):
            st = ST[s]
            tot = st["tot"]
            am = tiny.tile([1, 12], f32, name=f"am{s}", tag=f"am{s}")
            V.tensor_tensor(am[:, 0:3], tot[:, 5:8], tot[:, 1:4], Alu.subtract)
            V.tensor_tensor(am[:, 11:12], tot[:, 4:5], tot[:, 0:1], Alu.subtract)
            V.tensor_scalar(am[:, 11:12], am[:, 11:12], 1.0, 0.0, Alu.max, Alu.add)
            V.reciprocal(am[:, 10:11], am[:, 11:12])
            V.tensor_tensor(am[:, 0:3], am[:, 0:3], fbcast(am[:, 10:11], 3), Alu.mult)
            V.tensor_scalar(am[:, 9:10], tot[:, 0:1], -1.0, float(TOPN),
                            Alu.mult, Alu.add)
            V.tensor_tensor(am[:, 0:3], am[:, 0:3], fbcast(am[:, 9:10], 3), Alu.mult)
            V.tensor_tensor(am[:, 0:3], am[:, 0:3], tot[:, 1:4], Alu.add)
            V.tensor_scalar(am[:, 0:3], am[:, 0:3], 1.0 / TOPN, 0.0, Alu.mult, Alu.add)
            V.tensor_scalar(am[:, 3:6], am[:, 0:3], 1.0, 1.0, Alu.mult, Alu.add)
            V.reciprocal(am[:, 3:6], am[:, 3:6])
            V.tensor_scalar(am[:, 0:3], am[:, 0:3], 0.5, 0.5, Alu.mult, Alu.add)
            V.tensor_scalar(am[:, 6:9], am[:, 0:3], -1.0, 0.5, Alu.mult, Alu.add)
            st["am"] = am

        def f_chsc(s, k):
            st = ST[s]
            if "chsc" not in st:
                st["chsc"] = tiny.tile([128, 9], f32, name=f"chsc{s}",
                                       tag=f"chsc{s}")
            bp = psml.tile([128, 1], f32, name=f"chps{s}", tag=f"fold{s}")
            nc.tensor.matmul(bp[:], c_ones1x[:], st["am"][0:1, k:k + 1],
                             start=True, stop=True)
            nc.scalar.copy(st["chsc"][:, k:k + 1], bp[:])

        def f_p(s):
            st = ST[s]
            p = pp.tile([128, NW], bf16, name=f"p{s}", tag=f"p{s}")
            nc.scalar.activation(p[:], st["uh"][:, 0:NW], Act.Identity,
                                 bias=1.0, scale=-OMEGA)
            st["p"] = p

        # ---------------------------------------------------------- backend
        def backend(s, pre=None):
            st = ST[s]
            guid, p, chsc = st["guid"], st["p"], st["chsc"]
            # reload f32 x for the output stage (ready by the time it's used)
            xrld = []
            for chn in range(3):
                t = big.tile([128, NW], f32, name=f"xr{s}_{chn}", tag="xrld")
                nc.scalar.dma_start(out=cview(t)[:, :, :],
                                    in_=x_ext[s, chn].rearrange(
                                        "(c p) w -> p c w", p=128))
                xrld.append(t)

            Ip = srcp.tile([128, NW], bf16, name="Ip", tag="srcp")
            V.tensor_tensor(Ip[:], guid[:], p[:], Alu.mult)
            if pre is None:
                II = srcp.tile([128, NW], bf16, name="II", tag="srcp")
                nc.scalar.activation(II[:], guid[:], Act.Square)
                cum = pp.tile([128, NCHUNK * CUMW], f32, name="cum", tag="cum")
                cvz = cview(cum, CUMW)
                for c in range(NCHUNK):
                    V.memset(cvz[:, c, 0:41], 0.0)
                hbs = {}
                srcs = (("I", guid), ("p", p), ("Ip", Ip), ("II", II))
            else:
                II, cum, hbs = pre["II"], pre["cum"], {"I": pre["hbI"]}
                srcs = (("p", p), ("Ip", Ip), ("II", II))
            for nm, src_t in srcs:
                hb_t = boxes.tile([128, NW], f32r, name=f"hb{nm}", tag="boxes")
                hbox(hb_t, src_t, cum)
                hbs[nm] = hb_t
            means = {}
            for nm in ("I", "p", "Ip", "II"):
                mn = boxes.tile([128, NW], f32, name=f"mean{nm}", tag="boxes")
                vbox(mn, hbs[nm])
                means[nm] = mn
            mI, mp_, mIp, mII = means["I"], means["p"], means["Ip"], means["II"]

            tmp = abt.tile([128, NW], f32, name="tmp", tag="abt")
            V.tensor_tensor(tmp[:], mI[:], mp_[:], Alu.mult)
            cov = abt.tile([128, NW], f32, name="cov", tag="abt")
            V.tensor_tensor(cov[:], mIp[:], tmp[:], Alu.subtract)
            sq = abt.tile([128, NW], f32, name="sq", tag="abt")
            nc.scalar.activation(sq[:], mI[:], Act.Square)
            V.scalar_tensor_tensor(sq[:], mII[:], EPS, sq[:], Alu.add, Alu.subtract)
            rec = abt.tile([128, NW], f32, name="rec", tag="abt")
            V.reciprocal_approx_fast(out=rec[:], in_=sq[:])
            a_t = srcp.tile([128, NW], bf16, name="a_t", tag="srcp")
            V.tensor_tensor(a_t[:], cov[:], rec[:], Alu.mult)
            b_t = srcp.tile([128, NW], bf16, name="b_t", tag="srcp")
            V.tensor_tensor(b_t[:], a_t[:], mI[:], Alu.mult)
            V.tensor_tensor(b_t[:], mp_[:], b_t[:], Alu.subtract)

            hba = boxes.tile([128, NW], f32r, name="hba", tag="boxes")
            hbox(hba, a_t, cum)
            hbb = boxes.tile([128, NW], f32r, name="hbb", tag="boxes")
            hbox(hbb, b_t, cum)
            mean_a = boxes.tile([128, NW], f32, name="mean_a", tag="boxes")
            vbox(mean_a, hba)
            mean_b = boxes.tile([128, NW], f32, name="mean_b", tag="boxes")
            vbox(mean_b, hbb)

            T_t = abt.tile([128, NW], f32, name="T_t", tag="abt")
            V.tensor_tensor(T_t[:], mean_a[:], guid[:], Alu.mult)
            V.tensor_tensor(T_t[:], T_t[:], mean_b[:], Alu.add)
            rT = abt.tile([128, NW], f32, name="rT", tag="abt")
            V.reciprocal_approx_fast(out=rT[:], in_=T_t[:])

            for chn in range(3):
                d_t = abt.tile([128, NW], bf16, name=f"d{chn}", tag="dout", bufs=2)
                nc.scalar.activation(d_t[:], xrld[chn][:], Act.Identity,
                                     bias=chsc[:, 6 + chn:7 + chn], scale=0.5)
                V.tensor_tensor(d_t[:], d_t[:], rT[:], Alu.mult)
                V.tensor_scalar(d_t[:], d_t[:], chsc[:, chn:chn + 1], 0.0,
                                Alu.add, Alu.add)
                nc.gpsimd.dma_start(out=y_ext[s, chn].rearrange(
                                        "(c p) w -> p c w", p=128),
                                    in_=cview(d_t)[:, :, :])

        # ================================================== emission order
        f_load(0)
        f_load(1)
        f_dark_pools(0, second=False)
        f_dark_pools(1, second=False)
        guid_fill = [lambda: f_guid(0), lambda: f_guid(1), lambda: None,
                     lambda: None]
        for step in range(4):
            vp_shift(0, step)
            vp_shift(1, step)
            guid_fill[step]()
            vp_min(0, step)
            vp_min(1, step)
        f_secant_init(0)
        f_secant_init(1)
        junk = pp.tile([128, NW], bf16, name="junk", tag="w1")
        for which in (0, 1):
            f_count0(0, which)
            f_count0(1, which)
        for rnd in range(SECANT_ROUNDS):
            f_secant_round(0, rnd)
            f_secant_round(1, rnd)
        f_msums(0)
        f_msums(1)
        f_bandprep(0)
        f_bandprep(1)
        f_bandsums(0)
        f_bandsums(1)
        f_afold(0)
        f_afold(1)
        f_amath(0)
        f_amath(1)
        for k in range(9):
            f_chsc(0, k)
            f_chsc(1, k)
        f_dark_pools(0, second=True)
        f_dark_pools(1, second=True)
        # precompute backend(0)'s guidance-only pieces inside the vpool gaps
        II0 = srcp.tile([128, NW], bf16, name="II0", tag="srcp")
        cum = pp.tile([128, NCHUNK * CUMW], f32, name="cum", tag="cum")
        hbI0 = boxes.tile([128, NW], f32r, name="hbI0", tag="boxes")
        g0 = ST[0]["guid"]
        sv0, cv0 = cview(g0), cview(cum, CUMW)
        hv0 = cview(hbI0)

        def pre_step(step):
            if step == 0:
                nc.scalar.activation(II0[:], g0[:], Act.Square)
                for c in range(NCHUNK):
                    V.memset(cv0[:, c, 0:41], 0.0)
            elif step == 1:
                for c in (0, 1):
                    V.tensor_tensor_scan(cv0[:, c, 41:553], sv0[:, c, :],
                                         c_zeros[:], 0.0, Alu.add, Alu.add)
            elif step == 2:
                for c in (2, 3):
                    V.tensor_tensor_scan(cv0[:, c, 41:553], sv0[:, c, :],
                                         c_zeros[:], 0.0, Alu.add, Alu.add)
            else:
                for c in range(NCHUNK):
                    V.tensor_copy(cv0[:, c, 553:593], fbcast(cv0[:, c, 552:553], 40))
                V.tensor_tensor(hv0[:, :, :], cv0[:, :, 81:593], cv0[:, :, 0:512],
                                Alu.subtract)

        for step in range(4):
            vp_shift(0, step)
            vp_shift(1, step)
            pre_step(step)
            vp_min(0, step)
            vp_min(1, step)
        f_p(0)
        f_p(1)
        backend(0, pre={"II": II0, "cum": cum, "hbI": hbI0})
        backend(1)

    nc.compile()
    return nc


def _get_program():
    if "nc" not in _CACHE:
        _CACHE["nc"] = _build()
    return _CACHE["nc"]


def kernel(x: np.ndarray) -> np.ndarray:
    from concourse.bass_utils import run_bass_kernel_spmd
    x = np.ascontiguousarray(np.asarray(x, dtype=np.float32))
    assert x.shape == (16, 3, H, W), x.shape
    nc = _get_program()
    consts = _host_consts()
    in_maps = [{"x": x[2 * i:2 * i + 2], **consts} for i in range(8)]
    res = run_bass_kernel_spmd(nc, in_maps, list(range(8)))
    out = np.concatenate([res.results[i]["y"] for i in range(8)], axis=0)
    return out.astype(np.float32)
